# revision 20
# baseline (speedup 1.0000x reference)
"""Trainium2 Bass kernel for nn_AttnApproximator (GQA attention + RoPE +
per-head shift correction), sharded over 8 NeuronCores.

Sharding: tensor-parallel over heads (4 groups of 8 query heads / 2 KV
heads) x data-parallel over batch (B=2) -> 8 cores. Each core computes a
partial output contribution [S, Dm] (its heads' slice of the attn @ Wo
contraction); the host sums the 4 head-group partials per batch element.

v2 design: one fused per-slice pipeline so the PE never drains between
phases and the ACT-bound attention inner loop is padded with out-proj
matmuls:
  per slice n:
    A(n): k/v/q projections + RoPE from one streamed hst slice (fp16)
    B(n): causal attention, scores kept transposed [sk, sq]:
            scoresT = kT.T @ qT  (partial-N on the diagonal band)
            ex = exp(scale*scoresT) on ACT, triangular mask on DVE
            attnT += v.T @ ex  (PSUM accumulate)
            softmax sums accumulated on DVE (not PE), one tiny
            ones-matmul + DVE reciprocal per head
          out-proj matmuls of slice n-1 are interleaved as PE filler.
    C(n): out[sl, :] = attnT.T @ Wo, drained via ACT, fp16 DMA out
"""

import math
import numpy as np

import bass_rust
import concourse.bass as bass
import concourse.tile as tile
from concourse import mybir
from concourse import bass_utils
from concourse.vector_clock import ScopedClock
from contextlib import ExitStack

# ---------------------------------------------------------------- constants
B, S, DM = 2, 2048, 4096
H, KV, D = 32, 8, 128
N_CORES = 8
TP = 4                    # head groups
HQ = H // TP              # 8 q heads per core
HKV = KV // TP            # 2 kv heads per core
GROUPS = H // KV          # 4
THETA = 10000.0
SQ = 512                  # s-slice width
NSL = S // SQ             # 4 slices
NK = DM // 128            # 32 contraction tiles
NCH = 4                   # hst chunks per slice
KCH = NK // NCH           # 8 k-tiles per chunk
NJ = DM // 512            # out-proj column blocks
SCALE = 1.0 / math.sqrt(D)

F32 = mybir.dt.float32
F16 = mybir.dt.float16
AF = mybir.ActivationFunctionType
F16N = np.float16


# ------------------------------------------------- walrus drain-wait fixup
class SplitDrainTileContext(tile.TileContext):
    """This container's walrus rejects >1 sync wait on the SP tail-drain
    CTRL instruction; split the gathered waits onto chained SP nops."""

    MAX_WAITS = 1

    def _drain_and_barrier(self, tick_clock, wait_clock):
        nc = self.nc
        drain_inst = nc.sync.drain()
        wait_clock.add_sem_waits(
            drain_inst.ins, ScopedClock({None: tick_clock.global_clock})
        )
        si = drain_inst.ins.sync_info
        waits = list(si.on_wait) if si is not None else []
        mw = self.MAX_WAITS
        if len(waits) > mw:
            drain_inst.ins.sync_info = bass_rust.SyncInfo(
                on_wait=waits[:mw], on_update=list(si.on_update)
            )
            for k in range(mw, len(waits), mw):
                nop = nc.sync.nop(nofuse=True, hint="drain_wait_split")
                nop.ins.sync_info = bass_rust.SyncInfo(
                    on_wait=waits[k : k + mw], on_update=[]
                )
        nc.all_engine_barrier()
        assert self.sems is not None
        popped = nc._tile_sem_poison_stack.pop()
        assert popped is self._sem_poison
        nc.clear_and_free_semaphores(list(self.sems.allocated().values()))
        nc.all_engine_barrier()


def _split_excess_waits(nc):
    """This walrus accepts 1 sync wait per instruction (2 for
    EventSemaphore). Tile emits more; move the excess onto same-engine
    NoOp carriers inserted immediately before the over-limit instruction."""
    uid = 0
    for fn in nc.m.functions:
        for bb in fn.blocks:
            new, changed = [], False
            for inst in bb.instructions:
                si = inst.sync_info
                waits = list(si.on_wait) if si is not None else []
                cap = 2 if inst.opcode == "EventSemaphore" else 1
                if len(waits) > cap:
                    changed = True
                    for w in waits[:-cap]:
                        nop = mybir.InstNoOp(
                            name=f"I-wsplit-{uid}",
                            engine=inst.engine,
                            bass_nofuse=True,
                            sync_info=mybir.SyncInfo(on_wait=[w], on_update=[]),
                        )
                        uid += 1
                        new.append(nop)
                    inst.sync_info = bass_rust.SyncInfo(
                        on_wait=waits[-cap:], on_update=list(si.on_update))
                new.append(inst)
            if changed:
                bb.instructions = new


# ---------------------------------------------------------------- builder
def _rope(nc, tmp_pool, out_ap, in_ps, cos_sl, sin_sl):
    """out = in*cosT + swap_halves(in)*sinT_signed ; in_ps is PSUM f32.
    First op drains PSUM via ACT (fp16 out); the DVE math is all fp16."""
    q_sb = tmp_pool.tile([128, SQ], F16, tag="rope_q")
    nc.scalar.copy(q_sb[:], in_ps[:])
    sw = tmp_pool.tile([128, SQ], F16, tag="rope_sw")
    nc.vector.tensor_copy(sw[0:64, :], q_sb[64:128, :])
    nc.vector.tensor_copy(sw[64:128, :], q_sb[0:64, :])
    nc.vector.tensor_mul(sw[:], sw[:], sin_sl)
    t2 = tmp_pool.tile([128, SQ], F16, tag="rope_t2")
    nc.vector.tensor_mul(t2[:], q_sb[:], cos_sl)
    nc.vector.tensor_add(out_ap, t2[:], sw[:])


def build_kernel():
    nc = bass.Bass("TRN2", target_bir_lowering=False, debug=False,
                   num_devices=N_CORES)

    din = lambda n, shp, dt: nc.dram_tensor(n, shp, dt, kind="ExternalInput").ap()
    hsT_t = din("hsT_t", [NSL, NCH, 128, KCH, SQ], F16)
    wq_t = din("wq_t", [HQ, 128, NK, D], F16)
    wk_t = din("wk_t", [128, NK, HKV * D], F16)
    wv_t = din("wv_t", [128, NK, HKV * D], F16)
    wo_t = din("wo_t", [NJ, 128, HQ, 512], F16)
    wsq = din("wsq", [128, HQ, D], F16)
    wsk = din("wsk", [128, HQ, D], F16)
    cosT = din("cosT", [128, S], F16)
    sinsg = din("sinsg", [128, S], F16)
    mask_in = din("maskin", [128, 128], F16)
    out = nc.dram_tensor("out", [S, DM], F16, kind="ExternalOutput").ap()

    with SplitDrainTileContext(nc) as tc, ExitStack() as octx:
        # ---------------- sbuf pools ----------------
        pers = octx.enter_context(tc.tile_pool(name="pers", bufs=1))
        kT_sb = pers.tile([128, HKV, S], F16, tag="kT")            # 8KB/p
        v_sb = pers.tile([128, S // 128, HKV * D], F16, tag="v")   # 8KB/p
        qT_sb = pers.tile([128, HQ, S], F16, tag="qT")             # 32KB/p
        cos_sb = pers.tile([128, S], F16, tag="cos")               # 4KB/p
        sin_sb = pers.tile([128, S], F16, tag="sin")               # 4KB/p
        wsq_sb = pers.tile([128, HQ, D], F16, tag="wsq")           # 2KB/p
        wsk_sb = pers.tile([128, HQ, D], F16, tag="wsk")           # 2KB/p
        mask_sb = pers.tile([128, 128], F16, tag="mask")
        ones_sq = pers.tile([128, 128], F16, tag="ones_sq")

        wkv = octx.enter_context(tc.tile_pool(name="wkv", bufs=1))
        wk_sb = wkv.tile([128, NK, HKV * D], F16, tag="wk")        # 16KB/p
        wv_sb = wkv.tile([128, NK, HKV * D], F16, tag="wv")        # 16KB/p

        hst_pool = octx.enter_context(tc.tile_pool(name="hst", bufs=NCH))
        wq_pool = octx.enter_context(tc.tile_pool(name="wqp", bufs=2))
        wo_pool = octx.enter_context(tc.tile_pool(name="wop", bufs=2))
        attnT_pool = octx.enter_context(tc.tile_pool(name="attnT", bufs=2))
        ex_pool = octx.enter_context(tc.tile_pool(name="exp", bufs=8))
        acc_pool = octx.enter_context(tc.tile_pool(name="accp", bufs=2))
        rtmp = octx.enter_context(tc.tile_pool(name="rtmp", bufs=2))
        ftmp = octx.enter_context(tc.tile_pool(name="ftmp", bufs=2))
        ot_pool = octx.enter_context(tc.tile_pool(name="otp", bufs=6))

        p_sA = octx.enter_context(tc.tile_pool(name="p_sA", bufs=3, space="PSUM"))
        p_at = octx.enter_context(tc.tile_pool(name="p_at", bufs=3, space="PSUM"))
        psmall = octx.enter_context(tc.tile_pool(name="psm", bufs=2, space="PSUM"))

        # ---------------- startup loads ----------------
        nc.any.memset(ones_sq[:], 1.0)
        hst_c = [[None] * NCH for _ in range(NSL)]
        # PE p-state warm-up: the PE ramps 0.65->1.2->2.4GHz over ~3us of
        # continuous execution; it would otherwise sit idle ~5us waiting for
        # the first DMAs, then run the first chains at reduced clock. Dummy
        # matmuls on the resident ones tile pre-heat it at zero cost.
        ps_w = p_sA.tile([128, SQ], F32, tag="ps", name="ps_warm")
        for i in range(56):
            nc.tensor.matmul(ps_w[:, 0:128], ones_sq[:], ones_sq[:],
                             start=True, stop=True)
        for c in range(NCH):
            if c == 0:
                nc.sync.dma_start(wk_sb[:, 0:KCH // 2, :],
                                  wk_t[:, 0:KCH // 2, :])
                nc.sync.dma_start(wk_sb[:, KCH // 2:KCH, :],
                                  wk_t[:, KCH // 2:KCH, :])
            else:
                nc.sync.dma_start(wk_sb[:, c * KCH:(c + 1) * KCH, :],
                                  wk_t[:, c * KCH:(c + 1) * KCH, :])
            t = hst_pool.tile([128, KCH, SQ], F16, tag="hst")
            if c == 0:
                nc.sync.dma_start(t[:, 0:KCH // 2, :],
                                  hsT_t[0, c, :, 0:KCH // 2, :])
                nc.sync.dma_start(t[:, KCH // 2:KCH, :],
                                  hsT_t[0, c, :, KCH // 2:KCH, :])
            else:
                nc.sync.dma_start(t[:], hsT_t[0, c])
            hst_c[0][c] = t
        nc.sync.dma_start(wv_sb[:], wv_t[:, :, :])
        nc.sync.dma_start(cos_sb[:], cosT[:, :])
        nc.sync.dma_start(sin_sb[:], sinsg[:, :])
        nc.sync.dma_start(mask_sb[:], mask_in[:, :])
        nc.sync.dma_start(wsq_sb[:], wsq[:, :, :])
        nc.sync.dma_start(wsk_sb[:], wsk[:, :, :])

        # ---------------- phase A: projections for slice n ----------------
        def emit_q_chain(n, h):
            sl = slice(n * SQ, (n + 1) * SQ)
            hs = hst_c[n]
            wqc = wq_pool.tile([128, NK, D], F16, tag="wqc")
            nc.sync.dma_start(wqc[:], wq_t[h])
            ps = p_sA.tile([128, SQ], F32, tag="ps")
            for k in range(NK):
                nc.tensor.matmul(ps[:], wqc[:, k, :], hs[k // KCH][:, k % KCH, :],
                                 start=(k == 0), stop=(k == NK - 1))
            _rope(nc, rtmp, qT_sb[:, h, sl], ps,
                  cos_sb[:, sl], sin_sb[:, sl])

        def emit_A(n, q_heads=range(HQ)):
            sl = slice(n * SQ, (n + 1) * SQ)
            hs = hst_c[n]

            def hk(k):
                return hs[k // KCH][:, k % KCH, :]

            # both kv chains advance chunk-by-chunk so the startup DMA of
            # hst chunk c+1 overlaps 2*KCH matmuls on chunk c
            ps_k = [p_sA.tile([128, SQ], F32, tag="ps", name=f"ps_k{kv}")
                    for kv in range(HKV)]
            for k in range(NK):
                for kv in range(HKV):
                    nc.tensor.matmul(ps_k[kv][:],
                                     wk_sb[:, k, kv * D:(kv + 1) * D],
                                     hk(k), start=(k == 0), stop=(k == NK - 1))
            for kv in range(HKV):
                _rope(nc, rtmp, kT_sb[:, kv, sl], ps_k[kv],
                      cos_sb[:, sl], sin_sb[:, sl])
            for s4 in range(SQ // 128):
                ps = p_sA.tile([128, HKV * D], F32, tag="ps")
                for k in range(NK):
                    nc.tensor.matmul(
                        ps[:], hs[k // KCH][:, k % KCH, s4 * 128:(s4 + 1) * 128],
                        wv_sb[:, k, :], start=(k == 0), stop=(k == NK - 1))
                nc.scalar.copy(v_sb[:, n * 4 + s4, :], ps[:])
            for h in q_heads:
                emit_q_chain(n, h)

        # ---------------- deferred per-head epilogues ----------------
        epi_q = []   # heads awaiting sums-matmul + reciprocal
        fin_q = []   # heads awaiting delta/normalize/store

        def emit_epi():
            h, kv, n, ps_at, acc, attn_buf = epi_q.pop(0)
            # all-ones lhsT gives the softmax sums already broadcast to all
            # 128 partitions in ONE full-width matmul; 1/sums as exp(-ln(x))
            # on ACT (per-lane cost is identical to the [1,512] variant)
            ps_b = psmall.tile([128, SQ], F32, tag="sp")
            nc.tensor.matmul(ps_b[:], ones_sq[:], acc[:], start=True, stop=True)
            lns = ftmp.tile([128, SQ], F32, tag="lns", bufs=1)
            nc.scalar.activation(lns[:], ps_b[:], AF.Ln)
            bc_sb = ftmp.tile([128, SQ], F32, tag="bc_sb")
            nc.scalar.activation(bc_sb[:], lns[:], AF.Exp, scale=-1.0)
            fin_q.append((h, kv, n, ps_at, bc_sb, attn_buf))

        def emit_fin():
            h, kv, n, ps_at, bc_sb, attn_buf = fin_q.pop(0)
            sl = slice(n * SQ, (n + 1) * SQ)
            ps_dl = psmall.tile([128, SQ], F32, tag="sp")
            nc.tensor.matmul(ps_dl[:], wsq_sb[:, h, :], qT_sb[:, h, sl],
                             start=True, stop=False)
            nc.tensor.matmul(ps_dl[:], wsk_sb[:, h, :], kT_sb[:, kv, sl],
                             start=False, stop=True)
            t1 = ftmp.tile([128, SQ], F32, tag="t1")
            nc.vector.tensor_mul(t1[:], ps_at[:], bc_sb[:])
            nc.vector.tensor_add(attn_buf[:, h, :], t1[:], ps_dl[:])

        def pop_fillers(cstate, k):
            while cstate is not None and cstate[1] > 0 and k > 0:
                next(cstate[0])
                cstate[1] -= 1
                k -= 1

        # ---------------- phase C: out-proj step generator ----------------
        def make_C(n, attn_buf):
            def gen():
                wo_tiles = {}
                wo_tiles[0] = wo_pool.tile([128, HQ, 512], F16, tag="wo", name="wo_sb0")
                nc.sync.dma_start(wo_tiles[0][:], wo_t[0])
                for j in range(NJ):
                    wo_sb = wo_tiles.pop(j)
                    yield
                    for m in range(SQ // 128):
                        if m == 0 and j + 1 < NJ:
                            nxt = wo_pool.tile([128, HQ, 512], F16, tag="wo", name="wo_sbn")
                            nc.sync.dma_start(nxt[:], wo_t[j + 1])
                            wo_tiles[j + 1] = nxt
                        ps = psmall.tile([128, 512], F32, tag="sp")
                        for t2 in range(HQ):
                            nc.tensor.matmul(
                                ps[:], attn_buf[:, t2, m * 128:(m + 1) * 128],
                                wo_sb[:, t2, :],
                                start=(t2 == 0), stop=(t2 == HQ - 1))
                            yield
                        ot = ot_pool.tile([128, 512], F16, tag="ot")
                        nc.vector.tensor_copy(ot[:], ps[:])
                        gm = n * 4 + m
                        nc.sync.dma_start(
                            out[gm * 128:(gm + 1) * 128, j * 512:(j + 1) * 512],
                            ot[:])
                        yield
            return [gen(), NJ * (1 + 4 * (HQ + 1))]

        # ---------------- phase B: attention heads for slice n ----------------
        def emit_B_heads(n, cstate, boundary_cb=None):
            nT = 4 * (n + 1)
            attn_buf = attnT_pool.tile([128, HQ, SQ], F16, tag="attnT")
            if n + 1 < NSL:
                for c in range(NCH):
                    t = hst_pool.tile([128, KCH, SQ], F16, tag="hst")
                    nc.sync.dma_start(t[:], hsT_t[n + 1, c])
                    hst_c[n + 1][c] = t
            total_iters = HQ * nT
            nsteps = cstate[1] if cstate is not None else 0
            # consume ~85% of the filler steps during the heads; the rest
            # pad the slice-end epilogue flush
            budget = (nsteps * 17) // 20
            it = 0
            popped = 0
            for h in range(HQ):
                kv = h // GROUPS
                ps_at = p_at.tile([128, SQ], F32, tag="ps_at")
                acc = acc_pool.tile([128, SQ], F16, tag="acc")

                def emit_at(pt, pex, psub, last):
                    nc.tensor.matmul(ps_at[:, psub:SQ],
                                     v_sb[:, pt, kv * D:(kv + 1) * D],
                                     pex[:, psub:SQ],
                                     start=(pt == 0), stop=last)
                    if pt == 0:
                        nc.vector.tensor_copy(acc[:], pex[:])
                    else:
                        nc.vector.tensor_add(acc[:, psub:SQ],
                                             acc[:, psub:SQ], pex[:, psub:SQ])

                pend = None
                for t in range(nT):
                    sub = max(0, (t - 4 * n) * 128)
                    ps_sc = p_sA.tile([128, SQ], F32, tag="ps")
                    nc.tensor.matmul(
                        ps_sc[:, sub:SQ],
                        kT_sb[:, kv, t * 128:(t + 1) * 128],
                        qT_sb[:, h, n * SQ + sub:(n + 1) * SQ],
                        start=True, stop=True)
                    ex = ex_pool.tile([128, SQ], F16, tag="ex")
                    nc.scalar.activation(ex[:, sub:SQ], ps_sc[:, sub:SQ],
                                         AF.Exp, scale=SCALE)
                    if t >= 4 * n:
                        nc.vector.tensor_mul(ex[:, sub:sub + 128],
                                             ex[:, sub:sub + 128], mask_sb[:])
                    # at-mm runs one iteration behind its exp so the PE
                    # never waits on a just-issued ACT op
                    if pend is not None:
                        emit_at(*pend, last=False)
                    pend = (t, ex, sub)
                    if t == 2 and epi_q:
                        emit_epi()
                    if t == 3 and fin_q:
                        emit_fin()
                    it += 1
                    want = (budget * it) // total_iters
                    if want > popped:
                        pop_fillers(cstate, want - popped)
                        popped = want
                emit_at(*pend, last=True)
                epi_q.append((h, kv, n, ps_at, acc, attn_buf))
                if boundary_cb is not None:
                    boundary_cb(h)
            return attn_buf

        def emit_flush(cstate):
            # pad the dependency-fresh epilogue chains with filler matmuls
            pop_fillers(cstate, 8)
            while epi_q or fin_q:
                if epi_q:
                    emit_epi()
                pop_fillers(cstate, 8)
                if fin_q:
                    emit_fin()
                pop_fillers(cstate, 8)
            if cstate is not None:
                pop_fillers(cstate, cstate[1])

        # ---------------- fused main loop ----------------
        cstate = None
        emit_A(0)
        for n in range(NSL):
            attn_buf = emit_B_heads(n, cstate)
            emit_flush(cstate)
            cstate = make_C(n, attn_buf)
            if n + 1 < NSL:
                emit_A(n + 1)
        pop_fillers(cstate, cstate[1])

    _split_excess_waits(nc)
    return nc


# ---------------------------------------------------------------- host side
_CACHE = {}


def _prep_core_inputs(inputs, core):
    b, g = core // TP, core % TP
    hs = np.asarray(inputs["hidden_states"])[b]          # [S, DM] f32
    pos = np.asarray(inputs["position_ids"])[b]          # [S] int32
    Wq, Wk, Wv, Wo = (np.asarray(inputs[k]) for k in ("Wq", "Wk", "Wv", "Wo"))
    Ws_q, Ws_k = np.asarray(inputs["Ws_q"]), np.asarray(inputs["Ws_k"])

    qh0 = g * HQ                 # first global q head
    kv0 = g * HKV                # first global kv head

    inv_freq = 1.0 / (THETA ** (np.arange(0, D, 2, dtype=np.float64) / D))
    freqs = pos.astype(np.float64)[:, None] * inv_freq[None, :]   # [S, 64]
    cos = np.cos(freqs)
    sin = np.sin(freqs)
    cosT = np.ascontiguousarray(np.concatenate([cos, cos], axis=1).T).astype(F16N)
    sinsg = np.ascontiguousarray(np.concatenate([-sin, sin], axis=1).T).astype(F16N)

    ii = np.arange(128)[:, None]
    cc = np.arange(128)[None, :]
    mask128 = (cc >= ii).astype(F16N)

    # pre-tile into exact on-chip layouts (contiguous per-partition DMAs)
    hsT = hs.T.astype(F16N)                                    # [DM, S]
    hsT_t = np.ascontiguousarray(
        hsT.reshape(NCH, KCH, 128, NSL, SQ).transpose(3, 0, 2, 1, 4))
    wq_c = Wq[:, qh0 * D:(qh0 + HQ) * D].astype(F16N)          # [DM, 1024]
    wq_t = np.ascontiguousarray(
        wq_c.reshape(NK, 128, HQ, D).transpose(2, 1, 0, 3))    # [h, p, k, m]
    wk_c = Wk[:, kv0 * D:(kv0 + HKV) * D].astype(F16N)
    wk_t = np.ascontiguousarray(
        wk_c.reshape(NK, 128, HKV * D).transpose(1, 0, 2))     # [p, k, m]
    wv_c = Wv[:, kv0 * D:(kv0 + HKV) * D].astype(F16N)
    wv_t = np.ascontiguousarray(
        wv_c.reshape(NK, 128, HKV * D).transpose(1, 0, 2))
    wo_c = Wo[qh0 * D:(qh0 + HQ) * D, :].astype(F16N)          # [1024, DM]
    wo_t = np.ascontiguousarray(
        wo_c.reshape(HQ, 128, NJ, 512).transpose(2, 1, 0, 3))  # [j,p,t,m]
    wsq_t = np.ascontiguousarray(
        Ws_q[qh0:qh0 + HQ].transpose(1, 0, 2)).astype(F16N)    # [d, h, e]
    wsk_t = np.ascontiguousarray(
        Ws_k[qh0:qh0 + HQ].transpose(1, 0, 2)).astype(F16N)
    return {
        "hsT_t": hsT_t,
        "wq_t": wq_t,
        "wk_t": wk_t,
        "wv_t": wv_t,
        "wo_t": wo_t,
        "wsq": wsq_t,
        "wsk": wsk_t,
        "cosT": cosT,
        "sinsg": sinsg,
        "maskin": mask128,
    }


def run(inputs, trace=False):
    if "nc" not in _CACHE:
        _CACHE["nc"] = build_kernel()
    nc = _CACHE["nc"]
    in_maps = [_prep_core_inputs(inputs, c) for c in range(N_CORES)]
    res = bass_utils.run_bass_kernel_spmd(
        nc, in_maps, core_ids=list(range(N_CORES)), trace=trace)
    full = np.zeros((B, S, DM), dtype=np.float32)
    for c in range(N_CORES):
        full[c // TP] += res.results[c]["out"].astype(np.float32)
    return full, res


def kernel(**inputs) -> np.ndarray:
    full, _ = run(inputs, trace=False)
    return full


# revision 22
# speedup vs baseline: 1.0591x; 1.0591x over previous
"""Trainium2 Bass kernel for nn_AttnApproximator (GQA attention + RoPE +
per-head shift correction), sharded over 8 NeuronCores.

Sharding: tensor-parallel over heads (4 groups of 8 query heads / 2 KV
heads) x data-parallel over batch (B=2) -> 8 cores. Each core computes a
partial output contribution [S, Dm] (its heads' slice of the attn @ Wo
contraction); the host sums the 4 head-group partials per batch element.

v2 design: one fused per-slice pipeline so the PE never drains between
phases and the ACT-bound attention inner loop is padded with out-proj
matmuls:
  per slice n:
    A(n): k/v/q projections + RoPE from one streamed hst slice (fp16)
    B(n): causal attention, scores kept transposed [sk, sq]:
            scoresT = kT.T @ qT  (partial-N on the diagonal band)
            ex = exp(scale*scoresT) on ACT, triangular mask on DVE
            attnT += v.T @ ex  (PSUM accumulate)
            softmax sums accumulated on DVE (not PE), one tiny
            ones-matmul + DVE reciprocal per head
          out-proj matmuls of slice n-1 are interleaved as PE filler.
    C(n): out[sl, :] = attnT.T @ Wo, drained via ACT, fp16 DMA out
"""

import math
import numpy as np

import bass_rust
import concourse.bass as bass
import concourse.tile as tile
from concourse import mybir
from concourse import bass_utils
from concourse.vector_clock import ScopedClock
from contextlib import ExitStack

# ---------------------------------------------------------------- constants
B, S, DM = 2, 2048, 4096
H, KV, D = 32, 8, 128
N_CORES = 8
TP = 4                    # head groups
HQ = H // TP              # 8 q heads per core
HKV = KV // TP            # 2 kv heads per core
GROUPS = H // KV          # 4
THETA = 10000.0
SQ = 512                  # s-slice width
NSL = S // SQ             # 4 slices
NK = DM // 128            # 32 contraction tiles
NCH = 4                   # hst chunks per slice
KCH = NK // NCH           # 8 k-tiles per chunk
NJ = DM // 512            # out-proj column blocks
SCALE = 1.0 / math.sqrt(D)

F32 = mybir.dt.float32
F16 = mybir.dt.float16
AF = mybir.ActivationFunctionType
F16N = np.float16


# ------------------------------------------------- walrus drain-wait fixup
class SplitDrainTileContext(tile.TileContext):
    """This container's walrus rejects >1 sync wait on the SP tail-drain
    CTRL instruction; split the gathered waits onto chained SP nops."""

    MAX_WAITS = 1

    def _drain_and_barrier(self, tick_clock, wait_clock):
        nc = self.nc
        drain_inst = nc.sync.drain()
        wait_clock.add_sem_waits(
            drain_inst.ins, ScopedClock({None: tick_clock.global_clock})
        )
        si = drain_inst.ins.sync_info
        waits = list(si.on_wait) if si is not None else []
        mw = self.MAX_WAITS
        if len(waits) > mw:
            drain_inst.ins.sync_info = bass_rust.SyncInfo(
                on_wait=waits[:mw], on_update=list(si.on_update)
            )
            for k in range(mw, len(waits), mw):
                nop = nc.sync.nop(nofuse=True, hint="drain_wait_split")
                nop.ins.sync_info = bass_rust.SyncInfo(
                    on_wait=waits[k : k + mw], on_update=[]
                )
        nc.all_engine_barrier()
        assert self.sems is not None
        popped = nc._tile_sem_poison_stack.pop()
        assert popped is self._sem_poison
        nc.clear_and_free_semaphores(list(self.sems.allocated().values()))
        nc.all_engine_barrier()


def _split_excess_waits(nc):
    """This walrus accepts 1 sync wait per instruction (2 for
    EventSemaphore). Tile emits more; move the excess onto same-engine
    NoOp carriers inserted immediately before the over-limit instruction."""
    uid = 0
    for fn in nc.m.functions:
        for bb in fn.blocks:
            new, changed = [], False
            for inst in bb.instructions:
                si = inst.sync_info
                waits = list(si.on_wait) if si is not None else []
                cap = 2 if inst.opcode == "EventSemaphore" else 1
                if len(waits) > cap:
                    changed = True
                    for w in waits[:-cap]:
                        nop = mybir.InstNoOp(
                            name=f"I-wsplit-{uid}",
                            engine=inst.engine,
                            bass_nofuse=True,
                            sync_info=mybir.SyncInfo(on_wait=[w], on_update=[]),
                        )
                        uid += 1
                        new.append(nop)
                    inst.sync_info = bass_rust.SyncInfo(
                        on_wait=waits[-cap:], on_update=list(si.on_update))
                new.append(inst)
            if changed:
                bb.instructions = new


# ---------------------------------------------------------------- builder
def _rope(nc, tmp_pool, out_ap, in_ps, cos_sl, sin_sl):
    """out = in*cosT + swap_halves(in)*sinT_signed ; in_ps is PSUM f32.
    First op drains PSUM via ACT (fp16 out); the DVE math is all fp16."""
    q_sb = tmp_pool.tile([128, SQ], F16, tag="rope_q")
    nc.scalar.copy(q_sb[:], in_ps[:])
    sw = tmp_pool.tile([128, SQ], F16, tag="rope_sw")
    nc.vector.tensor_copy(sw[0:64, :], q_sb[64:128, :])
    nc.vector.tensor_copy(sw[64:128, :], q_sb[0:64, :])
    nc.vector.tensor_mul(sw[:], sw[:], sin_sl)
    t2 = tmp_pool.tile([128, SQ], F16, tag="rope_t2")
    nc.vector.tensor_mul(t2[:], q_sb[:], cos_sl)
    nc.vector.tensor_add(out_ap, t2[:], sw[:])


def build_kernel():
    nc = bass.Bass("TRN2", target_bir_lowering=False, debug=False,
                   num_devices=N_CORES)

    din = lambda n, shp, dt: nc.dram_tensor(n, shp, dt, kind="ExternalInput").ap()
    hsT_t = din("hsT_t", [NSL, NCH, 128, KCH, SQ], F16)
    wq_t = din("wq_t", [HQ, 128, NK, D], F16)
    wk_t = din("wk_t", [128, NK, HKV * D], F16)
    wv_t = din("wv_t", [128, NK, HKV * D], F16)
    wo_t = din("wo_t", [NJ, 128, HQ, 512], F16)
    wsq = din("wsq", [128, HQ, D], F16)
    wsk = din("wsk", [128, HQ, D], F16)
    cosT = din("cosT", [128, S], F16)
    sinsg = din("sinsg", [128, S], F16)
    mask_in = din("maskin", [128, 128], F16)
    out = nc.dram_tensor("out", [S, DM], F16, kind="ExternalOutput").ap()

    with SplitDrainTileContext(nc) as tc, ExitStack() as octx:
        # ---------------- sbuf pools ----------------
        pers = octx.enter_context(tc.tile_pool(name="pers", bufs=1))
        kT_sb = pers.tile([128, HKV, S], F16, tag="kT")            # 8KB/p
        v_sb = pers.tile([128, S // 128, HKV * D], F16, tag="v")   # 8KB/p
        qT_sb = pers.tile([128, HQ, S], F16, tag="qT")             # 32KB/p
        cos_sb = pers.tile([128, S], F16, tag="cos")               # 4KB/p
        sin_sb = pers.tile([128, S], F16, tag="sin")               # 4KB/p
        wsq_sb = pers.tile([128, HQ, D], F16, tag="wsq")           # 2KB/p
        wsk_sb = pers.tile([128, HQ, D], F16, tag="wsk")           # 2KB/p
        mask_sb = pers.tile([128, 128], F16, tag="mask")
        ones_sq = pers.tile([128, 128], F16, tag="ones_sq")

        wkv = octx.enter_context(tc.tile_pool(name="wkv", bufs=1))
        wk_sb = wkv.tile([128, NK, HKV * D], F16, tag="wk")        # 16KB/p
        wv_sb = wkv.tile([128, NK, HKV * D], F16, tag="wv")        # 16KB/p

        hst_pool = octx.enter_context(tc.tile_pool(name="hst", bufs=NCH))
        wq_pool = octx.enter_context(tc.tile_pool(name="wqp", bufs=2))
        wo_pool = octx.enter_context(tc.tile_pool(name="wop", bufs=2))
        attnT_pool = octx.enter_context(tc.tile_pool(name="attnT", bufs=2))
        ex_pool = octx.enter_context(tc.tile_pool(name="exp", bufs=8))
        acc_pool = octx.enter_context(tc.tile_pool(name="accp", bufs=2))
        rtmp = octx.enter_context(tc.tile_pool(name="rtmp", bufs=2))
        ftmp = octx.enter_context(tc.tile_pool(name="ftmp", bufs=2))
        ot_pool = octx.enter_context(tc.tile_pool(name="otp", bufs=6))

        p_sA = octx.enter_context(tc.tile_pool(name="p_sA", bufs=3, space="PSUM"))
        p_at = octx.enter_context(tc.tile_pool(name="p_at", bufs=3, space="PSUM"))
        psmall = octx.enter_context(tc.tile_pool(name="psm", bufs=2, space="PSUM"))

        # ---------------- startup loads ----------------
        nc.any.memset(ones_sq[:], 1.0)
        hst_c = [[None] * NCH for _ in range(NSL)]
        for c in range(NCH):
            if c == 0:
                nc.sync.dma_start(wk_sb[:, 0:KCH // 2, :],
                                  wk_t[:, 0:KCH // 2, :])
                nc.sync.dma_start(wk_sb[:, KCH // 2:KCH, :],
                                  wk_t[:, KCH // 2:KCH, :])
            else:
                nc.sync.dma_start(wk_sb[:, c * KCH:(c + 1) * KCH, :],
                                  wk_t[:, c * KCH:(c + 1) * KCH, :])
            t = hst_pool.tile([128, KCH, SQ], F16, tag="hst")
            if c == 0:
                nc.sync.dma_start(t[:, 0:KCH // 2, :],
                                  hsT_t[0, c, :, 0:KCH // 2, :])
                nc.sync.dma_start(t[:, KCH // 2:KCH, :],
                                  hsT_t[0, c, :, KCH // 2:KCH, :])
            else:
                nc.sync.dma_start(t[:], hsT_t[0, c])
            hst_c[0][c] = t
        nc.sync.dma_start(wv_sb[:], wv_t[:, :, :])
        nc.sync.dma_start(cos_sb[:], cosT[:, :])
        nc.sync.dma_start(sin_sb[:], sinsg[:, :])
        nc.sync.dma_start(mask_sb[:], mask_in[:, :])
        nc.sync.dma_start(wsq_sb[:], wsq[:, :, :])
        nc.sync.dma_start(wsk_sb[:], wsk[:, :, :])

        # ---------------- phase A: projections for slice n ----------------
        def emit_q_chain(n, h):
            sl = slice(n * SQ, (n + 1) * SQ)
            hs = hst_c[n]
            wqc = wq_pool.tile([128, NK, D], F16, tag="wqc")
            nc.sync.dma_start(wqc[:], wq_t[h])
            ps = p_sA.tile([128, SQ], F32, tag="ps")
            for k in range(NK):
                nc.tensor.matmul(ps[:], wqc[:, k, :], hs[k // KCH][:, k % KCH, :],
                                 start=(k == 0), stop=(k == NK - 1))
            _rope(nc, rtmp, qT_sb[:, h, sl], ps,
                  cos_sb[:, sl], sin_sb[:, sl])

        def emit_A(n, q_heads=range(HQ)):
            sl = slice(n * SQ, (n + 1) * SQ)
            hs = hst_c[n]

            def hk(k):
                return hs[k // KCH][:, k % KCH, :]

            # both kv chains advance chunk-by-chunk so the startup DMA of
            # hst chunk c+1 overlaps 2*KCH matmuls on chunk c
            ps_k = [p_sA.tile([128, SQ], F32, tag="ps", name=f"ps_k{kv}")
                    for kv in range(HKV)]
            for k in range(NK):
                for kv in range(HKV):
                    nc.tensor.matmul(ps_k[kv][:],
                                     wk_sb[:, k, kv * D:(kv + 1) * D],
                                     hk(k), start=(k == 0), stop=(k == NK - 1))
            for kv in range(HKV):
                _rope(nc, rtmp, kT_sb[:, kv, sl], ps_k[kv],
                      cos_sb[:, sl], sin_sb[:, sl])
            for s4 in range(SQ // 128):
                ps = p_sA.tile([128, HKV * D], F32, tag="ps")
                for k in range(NK):
                    nc.tensor.matmul(
                        ps[:], hs[k // KCH][:, k % KCH, s4 * 128:(s4 + 1) * 128],
                        wv_sb[:, k, :], start=(k == 0), stop=(k == NK - 1))
                nc.scalar.copy(v_sb[:, n * 4 + s4, :], ps[:])
            for h in q_heads:
                emit_q_chain(n, h)

        # ---------------- deferred per-head epilogues ----------------
        epi_q = []   # heads awaiting sums-matmul + reciprocal
        fin_q = []   # heads awaiting delta/normalize/store

        def emit_epi():
            h, kv, n, ps_at, acc, attn_buf = epi_q.pop(0)
            # all-ones lhsT gives the softmax sums already broadcast to all
            # 128 partitions in ONE full-width matmul; 1/sums as exp(-ln(x))
            # on ACT (per-lane cost is identical to the [1,512] variant)
            ps_b = psmall.tile([128, SQ], F32, tag="sp")
            nc.tensor.matmul(ps_b[:], ones_sq[:], acc[:], start=True, stop=True)
            lns = ftmp.tile([128, SQ], F32, tag="lns", bufs=1)
            nc.scalar.activation(lns[:], ps_b[:], AF.Ln)
            bc_sb = ftmp.tile([128, SQ], F32, tag="bc_sb")
            nc.scalar.activation(bc_sb[:], lns[:], AF.Exp, scale=-1.0)
            fin_q.append((h, kv, n, ps_at, bc_sb, attn_buf))

        def emit_fin():
            h, kv, n, ps_at, bc_sb, attn_buf = fin_q.pop(0)
            sl = slice(n * SQ, (n + 1) * SQ)
            ps_dl = psmall.tile([128, SQ], F32, tag="sp")
            nc.tensor.matmul(ps_dl[:], wsq_sb[:, h, :], qT_sb[:, h, sl],
                             start=True, stop=False)
            nc.tensor.matmul(ps_dl[:], wsk_sb[:, h, :], kT_sb[:, kv, sl],
                             start=False, stop=True)
            t1 = ftmp.tile([128, SQ], F32, tag="t1")
            nc.vector.tensor_mul(t1[:], ps_at[:], bc_sb[:])
            nc.vector.tensor_add(attn_buf[:, h, :], t1[:], ps_dl[:])

        def pop_fillers(cstate, k):
            while cstate is not None and cstate[1] > 0 and k > 0:
                next(cstate[0])
                cstate[1] -= 1
                k -= 1

        # ---------------- phase C: out-proj step generator ----------------
        def make_C(n, attn_buf):
            # issue the first wo load NOW (mid-flush): it overlaps the
            # slice-end epilogue padding instead of stalling C's first mms
            wo_tiles = {}
            wo_tiles[0] = wo_pool.tile([128, HQ, 512], F16, tag="wo", name="wo_sb0")
            nc.sync.dma_start(wo_tiles[0][:], wo_t[0])

            def gen():
                for j in range(NJ):
                    wo_sb = wo_tiles.pop(j)
                    yield
                    for m in range(SQ // 128):
                        if m == 0 and j + 1 < NJ:
                            nxt = wo_pool.tile([128, HQ, 512], F16, tag="wo", name="wo_sbn")
                            nc.sync.dma_start(nxt[:], wo_t[j + 1])
                            wo_tiles[j + 1] = nxt
                        ps = psmall.tile([128, 512], F32, tag="sp")
                        for t2 in range(HQ):
                            nc.tensor.matmul(
                                ps[:], attn_buf[:, t2, m * 128:(m + 1) * 128],
                                wo_sb[:, t2, :],
                                start=(t2 == 0), stop=(t2 == HQ - 1))
                            yield
                        ot = ot_pool.tile([128, 512], F16, tag="ot")
                        nc.vector.tensor_copy(ot[:], ps[:])
                        gm = n * 4 + m
                        nc.sync.dma_start(
                            out[gm * 128:(gm + 1) * 128, j * 512:(j + 1) * 512],
                            ot[:])
                        yield
            return [gen(), NJ * (1 + 4 * (HQ + 1))]

        # ---------------- phase B: attention heads for slice n ----------------
        def emit_B_heads(n, cstate, boundary_cb=None):
            nT = 4 * (n + 1)
            attn_buf = attnT_pool.tile([128, HQ, SQ], F16, tag="attnT")
            if n + 1 < NSL:
                for c in range(NCH):
                    t = hst_pool.tile([128, KCH, SQ], F16, tag="hst")
                    nc.sync.dma_start(t[:], hsT_t[n + 1, c])
                    hst_c[n + 1][c] = t
            total_iters = HQ * nT
            nsteps = cstate[1] if cstate is not None else 0
            # consume ~85% of the filler steps during the heads; the rest
            # pad the slice-end epilogue flush
            budget = (nsteps * 17) // 20
            it = 0
            popped = 0
            for h in range(HQ):
                kv = h // GROUPS
                ps_at = p_at.tile([128, SQ], F32, tag="ps_at")
                acc = acc_pool.tile([128, SQ], F16, tag="acc")

                def emit_at(pt, pex, psub, last):
                    nc.tensor.matmul(ps_at[:, psub:SQ],
                                     v_sb[:, pt, kv * D:(kv + 1) * D],
                                     pex[:, psub:SQ],
                                     start=(pt == 0), stop=last)
                    if pt == 0:
                        nc.vector.tensor_copy(acc[:], pex[:])
                    else:
                        nc.vector.tensor_add(acc[:, psub:SQ],
                                             acc[:, psub:SQ], pex[:, psub:SQ])

                pend = None
                for t in range(nT):
                    sub = max(0, (t - 4 * n) * 128)
                    ps_sc = p_sA.tile([128, SQ], F32, tag="ps")
                    nc.tensor.matmul(
                        ps_sc[:, sub:SQ],
                        kT_sb[:, kv, t * 128:(t + 1) * 128],
                        qT_sb[:, h, n * SQ + sub:(n + 1) * SQ],
                        start=True, stop=True)
                    ex = ex_pool.tile([128, SQ], F16, tag="ex")
                    nc.scalar.activation(ex[:, sub:SQ], ps_sc[:, sub:SQ],
                                         AF.Exp, scale=SCALE)
                    if t >= 4 * n:
                        nc.vector.tensor_mul(ex[:, sub:sub + 128],
                                             ex[:, sub:sub + 128], mask_sb[:])
                    # at-mm runs one iteration behind its exp so the PE
                    # never waits on a just-issued ACT op
                    if pend is not None:
                        emit_at(*pend, last=False)
                    pend = (t, ex, sub)
                    if t == 2 and epi_q:
                        emit_epi()
                    if t == 3 and fin_q:
                        emit_fin()
                    it += 1
                    want = (budget * it) // total_iters
                    if want > popped:
                        pop_fillers(cstate, want - popped)
                        popped = want
                emit_at(*pend, last=True)
                epi_q.append((h, kv, n, ps_at, acc, attn_buf))
                if boundary_cb is not None:
                    boundary_cb(h)
            return attn_buf

        def emit_flush(cstate):
            # pad the dependency-fresh epilogue chains with filler matmuls
            pop_fillers(cstate, 8)
            while epi_q or fin_q:
                if epi_q:
                    emit_epi()
                pop_fillers(cstate, 8)
                if fin_q:
                    emit_fin()
                pop_fillers(cstate, 8)
            if cstate is not None:
                pop_fillers(cstate, cstate[1])

        # ---------------- fused main loop ----------------
        cstate = None
        emit_A(0)
        for n in range(NSL):
            attn_buf = emit_B_heads(n, cstate)
            cstate_next = make_C(n, attn_buf)
            emit_flush(cstate)
            cstate = cstate_next
            if n + 1 < NSL:
                emit_A(n + 1)
        pop_fillers(cstate, cstate[1])

    _split_excess_waits(nc)
    return nc


# ---------------------------------------------------------------- host side
_CACHE = {}


def _prep_core_inputs(inputs, core):
    b, g = core // TP, core % TP
    hs = np.asarray(inputs["hidden_states"])[b]          # [S, DM] f32
    pos = np.asarray(inputs["position_ids"])[b]          # [S] int32
    Wq, Wk, Wv, Wo = (np.asarray(inputs[k]) for k in ("Wq", "Wk", "Wv", "Wo"))
    Ws_q, Ws_k = np.asarray(inputs["Ws_q"]), np.asarray(inputs["Ws_k"])

    qh0 = g * HQ                 # first global q head
    kv0 = g * HKV                # first global kv head

    inv_freq = 1.0 / (THETA ** (np.arange(0, D, 2, dtype=np.float64) / D))
    freqs = pos.astype(np.float64)[:, None] * inv_freq[None, :]   # [S, 64]
    cos = np.cos(freqs)
    sin = np.sin(freqs)
    cosT = np.ascontiguousarray(np.concatenate([cos, cos], axis=1).T).astype(F16N)
    sinsg = np.ascontiguousarray(np.concatenate([-sin, sin], axis=1).T).astype(F16N)

    ii = np.arange(128)[:, None]
    cc = np.arange(128)[None, :]
    mask128 = (cc >= ii).astype(F16N)

    # pre-tile into exact on-chip layouts (contiguous per-partition DMAs)
    hsT = hs.T.astype(F16N)                                    # [DM, S]
    hsT_t = np.ascontiguousarray(
        hsT.reshape(NCH, KCH, 128, NSL, SQ).transpose(3, 0, 2, 1, 4))
    wq_c = Wq[:, qh0 * D:(qh0 + HQ) * D].astype(F16N)          # [DM, 1024]
    wq_t = np.ascontiguousarray(
        wq_c.reshape(NK, 128, HQ, D).transpose(2, 1, 0, 3))    # [h, p, k, m]
    wk_c = Wk[:, kv0 * D:(kv0 + HKV) * D].astype(F16N)
    wk_t = np.ascontiguousarray(
        wk_c.reshape(NK, 128, HKV * D).transpose(1, 0, 2))     # [p, k, m]
    wv_c = Wv[:, kv0 * D:(kv0 + HKV) * D].astype(F16N)
    wv_t = np.ascontiguousarray(
        wv_c.reshape(NK, 128, HKV * D).transpose(1, 0, 2))
    wo_c = Wo[qh0 * D:(qh0 + HQ) * D, :].astype(F16N)          # [1024, DM]
    wo_t = np.ascontiguousarray(
        wo_c.reshape(HQ, 128, NJ, 512).transpose(2, 1, 0, 3))  # [j,p,t,m]
    wsq_t = np.ascontiguousarray(
        Ws_q[qh0:qh0 + HQ].transpose(1, 0, 2)).astype(F16N)    # [d, h, e]
    wsk_t = np.ascontiguousarray(
        Ws_k[qh0:qh0 + HQ].transpose(1, 0, 2)).astype(F16N)
    return {
        "hsT_t": hsT_t,
        "wq_t": wq_t,
        "wk_t": wk_t,
        "wv_t": wv_t,
        "wo_t": wo_t,
        "wsq": wsq_t,
        "wsk": wsk_t,
        "cosT": cosT,
        "sinsg": sinsg,
        "maskin": mask128,
    }


def run(inputs, trace=False):
    if "nc" not in _CACHE:
        _CACHE["nc"] = build_kernel()
    nc = _CACHE["nc"]
    in_maps = [_prep_core_inputs(inputs, c) for c in range(N_CORES)]
    res = bass_utils.run_bass_kernel_spmd(
        nc, in_maps, core_ids=list(range(N_CORES)), trace=trace)
    full = np.zeros((B, S, DM), dtype=np.float32)
    for c in range(N_CORES):
        full[c // TP] += res.results[c]["out"].astype(np.float32)
    return full, res


def kernel(**inputs) -> np.ndarray:
    full, _ = run(inputs, trace=False)
    return full


# revision 23
# speedup vs baseline: 1.0624x; 1.0031x over previous
"""Trainium2 Bass kernel for nn_AttnApproximator (GQA attention + RoPE +
per-head shift correction), sharded over 8 NeuronCores.

Sharding: tensor-parallel over heads (4 groups of 8 query heads / 2 KV
heads) x data-parallel over batch (B=2) -> 8 cores. Each core computes a
partial output contribution [S, Dm] (its heads' slice of the attn @ Wo
contraction); the host sums the 4 head-group partials per batch element.

v2 design: one fused per-slice pipeline so the PE never drains between
phases and the ACT-bound attention inner loop is padded with out-proj
matmuls:
  per slice n:
    A(n): k/v/q projections + RoPE from one streamed hst slice (fp16)
    B(n): causal attention, scores kept transposed [sk, sq]:
            scoresT = kT.T @ qT  (partial-N on the diagonal band)
            ex = exp(scale*scoresT) on ACT, triangular mask on DVE
            attnT += v.T @ ex  (PSUM accumulate)
            softmax sums accumulated on DVE (not PE), one tiny
            ones-matmul + DVE reciprocal per head
          out-proj matmuls of slice n-1 are interleaved as PE filler.
    C(n): out[sl, :] = attnT.T @ Wo, drained via ACT, fp16 DMA out
"""

import math
import numpy as np

import bass_rust
import concourse.bass as bass
import concourse.tile as tile
from concourse import mybir
from concourse import bass_utils
from concourse.vector_clock import ScopedClock
from contextlib import ExitStack

# ---------------------------------------------------------------- constants
B, S, DM = 2, 2048, 4096
H, KV, D = 32, 8, 128
N_CORES = 8
TP = 4                    # head groups
HQ = H // TP              # 8 q heads per core
HKV = KV // TP            # 2 kv heads per core
GROUPS = H // KV          # 4
THETA = 10000.0
SQ = 512                  # s-slice width
NSL = S // SQ             # 4 slices
NK = DM // 128            # 32 contraction tiles
NCH = 4                   # hst chunks per slice
KCH = NK // NCH           # 8 k-tiles per chunk
NJ = DM // 512            # out-proj column blocks
SCALE = 1.0 / math.sqrt(D)

F32 = mybir.dt.float32
F16 = mybir.dt.float16
AF = mybir.ActivationFunctionType
F16N = np.float16


# ------------------------------------------------- walrus drain-wait fixup
class SplitDrainTileContext(tile.TileContext):
    """This container's walrus rejects >1 sync wait on the SP tail-drain
    CTRL instruction; split the gathered waits onto chained SP nops."""

    MAX_WAITS = 1

    def _drain_and_barrier(self, tick_clock, wait_clock):
        nc = self.nc
        drain_inst = nc.sync.drain()
        wait_clock.add_sem_waits(
            drain_inst.ins, ScopedClock({None: tick_clock.global_clock})
        )
        si = drain_inst.ins.sync_info
        waits = list(si.on_wait) if si is not None else []
        mw = self.MAX_WAITS
        if len(waits) > mw:
            drain_inst.ins.sync_info = bass_rust.SyncInfo(
                on_wait=waits[:mw], on_update=list(si.on_update)
            )
            for k in range(mw, len(waits), mw):
                nop = nc.sync.nop(nofuse=True, hint="drain_wait_split")
                nop.ins.sync_info = bass_rust.SyncInfo(
                    on_wait=waits[k : k + mw], on_update=[]
                )
        nc.all_engine_barrier()
        assert self.sems is not None
        popped = nc._tile_sem_poison_stack.pop()
        assert popped is self._sem_poison
        nc.clear_and_free_semaphores(list(self.sems.allocated().values()))
        nc.all_engine_barrier()


def _split_excess_waits(nc):
    """This walrus accepts 1 sync wait per instruction (2 for
    EventSemaphore). Tile emits more; move the excess onto same-engine
    NoOp carriers inserted immediately before the over-limit instruction."""
    uid = 0
    for fn in nc.m.functions:
        for bb in fn.blocks:
            new, changed = [], False
            for inst in bb.instructions:
                si = inst.sync_info
                waits = list(si.on_wait) if si is not None else []
                cap = 2 if inst.opcode == "EventSemaphore" else 1
                if len(waits) > cap:
                    changed = True
                    for w in waits[:-cap]:
                        nop = mybir.InstNoOp(
                            name=f"I-wsplit-{uid}",
                            engine=inst.engine,
                            bass_nofuse=True,
                            sync_info=mybir.SyncInfo(on_wait=[w], on_update=[]),
                        )
                        uid += 1
                        new.append(nop)
                    inst.sync_info = bass_rust.SyncInfo(
                        on_wait=waits[-cap:], on_update=list(si.on_update))
                new.append(inst)
            if changed:
                bb.instructions = new


# ---------------------------------------------------------------- builder
def _rope(nc, tmp_pool, out_ap, in_ps, cos_sl, sin_sl):
    """out = in*cosT + swap_halves(in)*sinT_signed ; in_ps is PSUM f32.
    First op drains PSUM via ACT (fp16 out); the DVE math is all fp16."""
    q_sb = tmp_pool.tile([128, SQ], F16, tag="rope_q")
    nc.scalar.copy(q_sb[:], in_ps[:])
    sw = tmp_pool.tile([128, SQ], F16, tag="rope_sw")
    nc.vector.tensor_copy(sw[0:64, :], q_sb[64:128, :])
    nc.vector.tensor_copy(sw[64:128, :], q_sb[0:64, :])
    nc.vector.tensor_mul(sw[:], sw[:], sin_sl)
    t2 = tmp_pool.tile([128, SQ], F16, tag="rope_t2")
    nc.vector.tensor_mul(t2[:], q_sb[:], cos_sl)
    nc.vector.tensor_add(out_ap, t2[:], sw[:])


def build_kernel():
    nc = bass.Bass("TRN2", target_bir_lowering=False, debug=False,
                   num_devices=N_CORES)

    din = lambda n, shp, dt: nc.dram_tensor(n, shp, dt, kind="ExternalInput").ap()
    hsT_t = din("hsT_t", [NSL, NCH, 128, KCH, SQ], F16)
    wq_t = din("wq_t", [HQ, 128, NK, D], F16)
    wk_t = din("wk_t", [128, NK, HKV * D], F16)
    wv_t = din("wv_t", [128, NK, HKV * D], F16)
    wo_t = din("wo_t", [NJ, 128, HQ, 512], F16)
    wsq = din("wsq", [128, HQ, D], F16)
    wsk = din("wsk", [128, HQ, D], F16)
    cosT = din("cosT", [128, S], F16)
    sinsg = din("sinsg", [128, S], F16)
    mask_in = din("maskin", [128, 128], F16)
    out = nc.dram_tensor("out", [S, DM], F16, kind="ExternalOutput").ap()

    with SplitDrainTileContext(nc) as tc, ExitStack() as octx:
        # ---------------- sbuf pools ----------------
        pers = octx.enter_context(tc.tile_pool(name="pers", bufs=1))
        kT_sb = pers.tile([128, HKV, S], F16, tag="kT")            # 8KB/p
        v_sb = pers.tile([128, S // 128, HKV * D], F16, tag="v")   # 8KB/p
        qT_sb = pers.tile([128, HQ, S], F16, tag="qT")             # 32KB/p
        cos_sb = pers.tile([128, S], F16, tag="cos")               # 4KB/p
        sin_sb = pers.tile([128, S], F16, tag="sin")               # 4KB/p
        wsq_sb = pers.tile([128, HQ, D], F16, tag="wsq")           # 2KB/p
        wsk_sb = pers.tile([128, HQ, D], F16, tag="wsk")           # 2KB/p
        mask_sb = pers.tile([128, 128], F16, tag="mask")
        ones_sq = pers.tile([128, 128], F16, tag="ones_sq")

        wkv = octx.enter_context(tc.tile_pool(name="wkv", bufs=1))
        wk_sb = wkv.tile([128, NK, HKV * D], F16, tag="wk")        # 16KB/p
        wv_sb = wkv.tile([128, NK, HKV * D], F16, tag="wv")        # 16KB/p

        hst_pool = octx.enter_context(tc.tile_pool(name="hst", bufs=NCH))
        wq_pool = octx.enter_context(tc.tile_pool(name="wqp", bufs=2))
        wo_pool = octx.enter_context(tc.tile_pool(name="wop", bufs=2))
        attnT_pool = octx.enter_context(tc.tile_pool(name="attnT", bufs=2))
        ex_pool = octx.enter_context(tc.tile_pool(name="exp", bufs=8))
        acc_pool = octx.enter_context(tc.tile_pool(name="accp", bufs=2))
        rtmp = octx.enter_context(tc.tile_pool(name="rtmp", bufs=2))
        ftmp = octx.enter_context(tc.tile_pool(name="ftmp", bufs=2))
        ot_pool = octx.enter_context(tc.tile_pool(name="otp", bufs=6))

        p_sA = octx.enter_context(tc.tile_pool(name="p_sA", bufs=3, space="PSUM"))
        p_at = octx.enter_context(tc.tile_pool(name="p_at", bufs=3, space="PSUM"))
        psmall = octx.enter_context(tc.tile_pool(name="psm", bufs=2, space="PSUM"))

        # ---------------- startup loads ----------------
        nc.any.memset(ones_sq[:], 1.0)
        hst_c = [[None] * NCH for _ in range(NSL)]
        for c in range(NCH):
            if c == 0:
                nc.sync.dma_start(wk_sb[:, 0:KCH // 2, :],
                                  wk_t[:, 0:KCH // 2, :])
                nc.sync.dma_start(wk_sb[:, KCH // 2:KCH, :],
                                  wk_t[:, KCH // 2:KCH, :])
            else:
                nc.sync.dma_start(wk_sb[:, c * KCH:(c + 1) * KCH, :],
                                  wk_t[:, c * KCH:(c + 1) * KCH, :])
            t = hst_pool.tile([128, KCH, SQ], F16, tag="hst")
            if c == 0:
                nc.sync.dma_start(t[:, 0:KCH // 2, :],
                                  hsT_t[0, c, :, 0:KCH // 2, :])
                nc.sync.dma_start(t[:, KCH // 2:KCH, :],
                                  hsT_t[0, c, :, KCH // 2:KCH, :])
            else:
                nc.sync.dma_start(t[:], hsT_t[0, c])
            hst_c[0][c] = t
        nc.sync.dma_start(wv_sb[:], wv_t[:, :, :])
        nc.sync.dma_start(cos_sb[:], cosT[:, :])
        nc.sync.dma_start(sin_sb[:], sinsg[:, :])
        nc.sync.dma_start(mask_sb[:], mask_in[:, :])
        nc.sync.dma_start(wsq_sb[:], wsq[:, :, :])
        nc.sync.dma_start(wsk_sb[:], wsk[:, :, :])

        # ---------------- phase A: projections for slice n ----------------
        def emit_q_chain(n, h):
            sl = slice(n * SQ, (n + 1) * SQ)
            hs = hst_c[n]
            wqc = wq_pool.tile([128, NK, D], F16, tag="wqc")
            nc.sync.dma_start(wqc[:], wq_t[h])
            ps = p_sA.tile([128, SQ], F32, tag="ps")
            for k in range(NK):
                nc.tensor.matmul(ps[:], wqc[:, k, :], hs[k // KCH][:, k % KCH, :],
                                 start=(k == 0), stop=(k == NK - 1))
            _rope(nc, rtmp, qT_sb[:, h, sl], ps,
                  cos_sb[:, sl], sin_sb[:, sl])

        def emit_A(n, q_heads=range(HQ)):
            sl = slice(n * SQ, (n + 1) * SQ)
            hs = hst_c[n]

            def hk(k):
                return hs[k // KCH][:, k % KCH, :]

            # both kv chains advance chunk-by-chunk so the startup DMA of
            # hst chunk c+1 overlaps 2*KCH matmuls on chunk c
            ps_k = [p_sA.tile([128, SQ], F32, tag="ps", name=f"ps_k{kv}")
                    for kv in range(HKV)]
            for k in range(NK):
                for kv in range(HKV):
                    nc.tensor.matmul(ps_k[kv][:],
                                     wk_sb[:, k, kv * D:(kv + 1) * D],
                                     hk(k), start=(k == 0), stop=(k == NK - 1))
            for kv in range(HKV):
                _rope(nc, rtmp, kT_sb[:, kv, sl], ps_k[kv],
                      cos_sb[:, sl], sin_sb[:, sl])
            for s4 in range(SQ // 128):
                ps = p_sA.tile([128, HKV * D], F32, tag="ps")
                for k in range(NK):
                    nc.tensor.matmul(
                        ps[:], hs[k // KCH][:, k % KCH, s4 * 128:(s4 + 1) * 128],
                        wv_sb[:, k, :], start=(k == 0), stop=(k == NK - 1))
                nc.scalar.copy(v_sb[:, n * 4 + s4, :], ps[:])
            for h in q_heads:
                emit_q_chain(n, h)

        # ---------------- deferred per-head epilogues ----------------
        epi_q = []   # heads awaiting sums-matmul + reciprocal
        fin_q = []   # heads awaiting delta/normalize/store

        def emit_epi():
            h, kv, n, ps_at, acc, attn_buf = epi_q.pop(0)
            # all-ones lhsT gives the softmax sums already broadcast to all
            # 128 partitions in ONE full-width matmul; 1/sums as exp(-ln(x))
            # on ACT (per-lane cost is identical to the [1,512] variant)
            ps_b = psmall.tile([128, SQ], F32, tag="sp")
            nc.tensor.matmul(ps_b[:], ones_sq[:], acc[:], start=True, stop=True)
            lns = ftmp.tile([128, SQ], F32, tag="lns", bufs=1)
            nc.scalar.activation(lns[:], ps_b[:], AF.Ln)
            bc_sb = ftmp.tile([128, SQ], F32, tag="bc_sb")
            nc.scalar.activation(bc_sb[:], lns[:], AF.Exp, scale=-1.0)
            fin_q.append((h, kv, n, ps_at, bc_sb, attn_buf))

        def emit_fin():
            h, kv, n, ps_at, bc_sb, attn_buf = fin_q.pop(0)
            sl = slice(n * SQ, (n + 1) * SQ)
            ps_dl = psmall.tile([128, SQ], F32, tag="sp")
            nc.tensor.matmul(ps_dl[:], wsq_sb[:, h, :], qT_sb[:, h, sl],
                             start=True, stop=False)
            nc.tensor.matmul(ps_dl[:], wsk_sb[:, h, :], kT_sb[:, kv, sl],
                             start=False, stop=True)
            t1 = ftmp.tile([128, SQ], F32, tag="t1")
            nc.vector.tensor_mul(t1[:], ps_at[:], bc_sb[:])
            nc.vector.tensor_add(attn_buf[:, h, :], t1[:], ps_dl[:])

        def pop_fillers(cstate, k):
            while cstate is not None and cstate[1] > 0 and k > 0:
                next(cstate[0])
                cstate[1] -= 1
                k -= 1

        # ---------------- phase C: out-proj step generator ----------------
        def make_C(n, attn_buf):
            def gen():
                wo_tiles = {}
                wo_tiles[0] = wo_pool.tile([128, HQ, 512], F16, tag="wo", name="wo_sb0")
                nc.sync.dma_start(wo_tiles[0][:], wo_t[0])
                for j in range(NJ):
                    wo_sb = wo_tiles.pop(j)
                    yield
                    for m in range(SQ // 128):
                        if m == 0 and j + 1 < NJ:
                            nxt = wo_pool.tile([128, HQ, 512], F16, tag="wo", name="wo_sbn")
                            nc.sync.dma_start(nxt[:], wo_t[j + 1])
                            wo_tiles[j + 1] = nxt
                        ps = psmall.tile([128, 512], F32, tag="sp")
                        for t2 in range(HQ):
                            nc.tensor.matmul(
                                ps[:], attn_buf[:, t2, m * 128:(m + 1) * 128],
                                wo_sb[:, t2, :],
                                start=(t2 == 0), stop=(t2 == HQ - 1))
                            yield
                        ot = ot_pool.tile([128, 512], F16, tag="ot")
                        nc.vector.tensor_copy(ot[:], ps[:])
                        gm = n * 4 + m
                        nc.sync.dma_start(
                            out[gm * 128:(gm + 1) * 128, j * 512:(j + 1) * 512],
                            ot[:])
                        yield
            return [gen(), NJ * (1 + 4 * (HQ + 1))]

        # ---------------- phase B: attention heads for slice n ----------------
        def emit_B_heads(n, cstate, boundary_cb=None):
            nT = 4 * (n + 1)
            attn_buf = attnT_pool.tile([128, HQ, SQ], F16, tag="attnT")
            if n + 1 < NSL:
                for c in range(NCH):
                    t = hst_pool.tile([128, KCH, SQ], F16, tag="hst")
                    nc.sync.dma_start(t[:], hsT_t[n + 1, c])
                    hst_c[n + 1][c] = t
            total_iters = HQ * nT
            nsteps = cstate[1] if cstate is not None else 0
            # consume ~85% of the filler steps during the heads; the rest
            # pad the slice-end epilogue flush
            budget = (nsteps * 17) // 20
            it = 0
            popped = 0
            for h in range(HQ):
                kv = h // GROUPS
                ps_at = p_at.tile([128, SQ], F32, tag="ps_at")
                acc = acc_pool.tile([128, SQ], F16, tag="acc")

                def emit_at(pt, pex, psub, last):
                    nc.tensor.matmul(ps_at[:, psub:SQ],
                                     v_sb[:, pt, kv * D:(kv + 1) * D],
                                     pex[:, psub:SQ],
                                     start=(pt == 0), stop=last)
                    if pt == 0:
                        nc.vector.tensor_copy(acc[:], pex[:])
                    else:
                        nc.vector.tensor_add(acc[:, psub:SQ],
                                             acc[:, psub:SQ], pex[:, psub:SQ])

                pend = None
                for t in range(nT):
                    sub = max(0, (t - 4 * n) * 128)
                    ps_sc = p_sA.tile([128, SQ], F32, tag="ps")
                    nc.tensor.matmul(
                        ps_sc[:, sub:SQ],
                        kT_sb[:, kv, t * 128:(t + 1) * 128],
                        qT_sb[:, h, n * SQ + sub:(n + 1) * SQ],
                        start=True, stop=True)
                    ex = ex_pool.tile([128, SQ], F16, tag="ex")
                    nc.scalar.activation(ex[:, sub:SQ], ps_sc[:, sub:SQ],
                                         AF.Exp, scale=SCALE)
                    if t >= 4 * n:
                        nc.vector.tensor_mul(ex[:, sub:sub + 128],
                                             ex[:, sub:sub + 128], mask_sb[:])
                    # at-mm runs one iteration behind its exp so the PE
                    # never waits on a just-issued ACT op
                    if pend is not None:
                        emit_at(*pend, last=False)
                    pend = (t, ex, sub)
                    if t == 2 and epi_q:
                        emit_epi()
                    if t == 3 and fin_q:
                        emit_fin()
                    it += 1
                    want = (budget * it) // total_iters
                    if want > popped:
                        pop_fillers(cstate, want - popped)
                        popped = want
                emit_at(*pend, last=True)
                epi_q.append((h, kv, n, ps_at, acc, attn_buf))
                if boundary_cb is not None:
                    boundary_cb(h)
            return attn_buf

        def emit_flush(cstate):
            # pad the dependency-fresh epilogue chains with filler matmuls
            pop_fillers(cstate, 8)
            while epi_q or fin_q:
                if epi_q:
                    emit_epi()
                pop_fillers(cstate, 8)
                if fin_q:
                    emit_fin()
                pop_fillers(cstate, 8)
            if cstate is not None:
                pop_fillers(cstate, cstate[1])

        # ---------------- fused main loop ----------------
        cstate = None
        emit_A(0)
        for n in range(NSL):
            attn_buf = emit_B_heads(n, cstate)
            emit_flush(cstate)
            cstate = make_C(n, attn_buf)
            if n + 1 < NSL:
                emit_A(n + 1)
        pop_fillers(cstate, cstate[1])

    _split_excess_waits(nc)
    return nc


# ---------------------------------------------------------------- host side
_CACHE = {}


def _prep_core_inputs(inputs, core):
    b, g = core // TP, core % TP
    hs = np.asarray(inputs["hidden_states"])[b]          # [S, DM] f32
    pos = np.asarray(inputs["position_ids"])[b]          # [S] int32
    Wq, Wk, Wv, Wo = (np.asarray(inputs[k]) for k in ("Wq", "Wk", "Wv", "Wo"))
    Ws_q, Ws_k = np.asarray(inputs["Ws_q"]), np.asarray(inputs["Ws_k"])

    qh0 = g * HQ                 # first global q head
    kv0 = g * HKV                # first global kv head

    inv_freq = 1.0 / (THETA ** (np.arange(0, D, 2, dtype=np.float64) / D))
    freqs = pos.astype(np.float64)[:, None] * inv_freq[None, :]   # [S, 64]
    cos = np.cos(freqs)
    sin = np.sin(freqs)
    cosT = np.ascontiguousarray(np.concatenate([cos, cos], axis=1).T).astype(F16N)
    sinsg = np.ascontiguousarray(np.concatenate([-sin, sin], axis=1).T).astype(F16N)

    ii = np.arange(128)[:, None]
    cc = np.arange(128)[None, :]
    mask128 = (cc >= ii).astype(F16N)

    # pre-tile into exact on-chip layouts (contiguous per-partition DMAs)
    hsT = hs.T.astype(F16N)                                    # [DM, S]
    hsT_t = np.ascontiguousarray(
        hsT.reshape(NCH, KCH, 128, NSL, SQ).transpose(3, 0, 2, 1, 4))
    wq_c = Wq[:, qh0 * D:(qh0 + HQ) * D].astype(F16N)          # [DM, 1024]
    wq_t = np.ascontiguousarray(
        wq_c.reshape(NK, 128, HQ, D).transpose(2, 1, 0, 3))    # [h, p, k, m]
    wk_c = Wk[:, kv0 * D:(kv0 + HKV) * D].astype(F16N)
    wk_t = np.ascontiguousarray(
        wk_c.reshape(NK, 128, HKV * D).transpose(1, 0, 2))     # [p, k, m]
    wv_c = Wv[:, kv0 * D:(kv0 + HKV) * D].astype(F16N)
    wv_t = np.ascontiguousarray(
        wv_c.reshape(NK, 128, HKV * D).transpose(1, 0, 2))
    wo_c = Wo[qh0 * D:(qh0 + HQ) * D, :].astype(F16N)          # [1024, DM]
    wo_t = np.ascontiguousarray(
        wo_c.reshape(HQ, 128, NJ, 512).transpose(2, 1, 0, 3))  # [j,p,t,m]
    wsq_t = np.ascontiguousarray(
        Ws_q[qh0:qh0 + HQ].transpose(1, 0, 2)).astype(F16N)    # [d, h, e]
    wsk_t = np.ascontiguousarray(
        Ws_k[qh0:qh0 + HQ].transpose(1, 0, 2)).astype(F16N)
    return {
        "hsT_t": hsT_t,
        "wq_t": wq_t,
        "wk_t": wk_t,
        "wv_t": wv_t,
        "wo_t": wo_t,
        "wsq": wsq_t,
        "wsk": wsk_t,
        "cosT": cosT,
        "sinsg": sinsg,
        "maskin": mask128,
    }


def run(inputs, trace=False):
    if "nc" not in _CACHE:
        _CACHE["nc"] = build_kernel()
    nc = _CACHE["nc"]
    in_maps = [_prep_core_inputs(inputs, c) for c in range(N_CORES)]
    res = bass_utils.run_bass_kernel_spmd(
        nc, in_maps, core_ids=list(range(N_CORES)), trace=trace)
    full = np.zeros((B, S, DM), dtype=np.float32)
    for c in range(N_CORES):
        full[c // TP] += res.results[c]["out"].astype(np.float32)
    return full, res


def kernel(**inputs) -> np.ndarray:
    full, _ = run(inputs, trace=False)
    return full
